# revision 13
# baseline (speedup 1.0000x reference)
"""Trainium2 Bass kernel for nn_DigitCapsuleLayer (dynamic-routing capsule layer).

Strategy
--------
Data-parallel over batch (8 cores x 32). u_hat (189 MB full) is never
materialized: every consumer is re-expressed as a matmul against the
(ri)=(r,i)-flattened operands, so per-core HBM traffic is just u + W.

Per routing iteration (3 total; the 3rd iteration's b-update is dead code
and skipped, so only 2 AllReduces):
  c_ij   = softmax(b) over R            (fp32; PE partition-sums + DVE)
  Wc     = W * c_ij                     (DVE + GpSimd, bf16)
  s      = uT^T @ Wc                    (PE, K=9216 chain, out [b, co])
  v      = squash(s)                    (ACT+DVE, fp32)
  T      = u_b^T @ v                    (PE per ri-tile, fp32 PSUM)
  a      = E^T @ (reduce_o(W*T))        (DVE mul+reduce, PE group-reduce)
  b     += AllReduce_batchmean(a)       (23 KB bf16 collective)

All matmul operands are bf16 (fp32 PSUM accumulation); the routing state
b/c/softmax/squash stays fp32. Measured end-to-end relative error ~3e-3.
"""

import sys

sys.path.insert(0, "/opt/trn_rl_repo")

import numpy as np
import ml_dtypes

import concourse.bass as bass
import concourse.tile as tile
from concourse import mybir
from concourse.bass_utils import run_bass_kernel_spmd
from concourse.vector_clock import ScopedClock

# ----------------------------------------------------------------------------
# Walrus workarounds: this image's walrus rejects any instruction carrying
# more than one sync wait. Split Tile's tail-drain waits and any other
# multi-wait instruction into single-wait NOPs on the same engine.
# ----------------------------------------------------------------------------

_uid = [0]


def _patched_drain_and_barrier(self, tick_clock, wait_clock):
    nc = self.nc
    probe = nc.sync.nop(nofuse=True, hint="tail_drain_waits")
    wait_clock.add_sem_waits(probe.ins, ScopedClock({None: tick_clock.global_clock}))
    si = probe.ins.sync_info
    waits = list(si.on_wait) if si is not None else []
    probe.ins.sync_info = mybir.SyncInfo(on_wait=waits[:1], on_update=[])
    for w in waits[1:]:
        n = nc.sync.nop(nofuse=True, hint="tail_drain_waits")
        n.ins.sync_info = mybir.SyncInfo(on_wait=[w], on_update=[])
    nc.sync.drain()
    nc.all_engine_barrier(sem_only=True)
    assert self.sems is not None
    popped = nc._tile_sem_poison_stack.pop()
    assert popped is self._sem_poison
    nc.clear_and_free_semaphores(list(self.sems.allocated().values()))


tile.TileContext._drain_and_barrier = _patched_drain_and_barrier


def _legalize_sync_waits(nc):
    for fn in nc.m.functions:
        for bb in fn.blocks:
            insts = bb.instructions
            i = 0
            while i < len(insts):
                inst = insts[i]
                si = getattr(inst, "sync_info", None)
                waits = list(si.on_wait) if si is not None else []
                if len(waits) > 1:
                    for w in waits[:-1]:
                        _uid[0] += 1
                        nop = mybir.InstNoOp(
                            name=f"I-waitsplit-{_uid[0]}", ins=[], outs=[]
                        )
                        nop.engine = inst.engine
                        nop.sync_info = mybir.SyncInfo(on_wait=[w], on_update=[])
                        insts.insert(i, nop)
                        i += 1
                    inst.sync_info = mybir.SyncInfo(
                        on_wait=[waits[-1]], on_update=list(si.on_update)
                    )
                i += 1


# ----------------------------------------------------------------------------
# Problem constants (hardcoded per contest contract)
# ----------------------------------------------------------------------------

B, R, C, O, I = 256, 1152, 10, 16, 8
NUM_ITERS = 3
N_CORES = 8
B_LOC = B // N_CORES          # 32
RI = R * I                    # 9216
CO = C * O                    # 160
NT = RI // 128                # 72 ri-tiles
NCHUNK = 8                    # ri-tiles per load/scale chunk
TB = 3                        # T-matmul tiles packed per PSUM bank
F32 = mybir.dt.float32
BF16 = mybir.dt.bfloat16
bfnp = ml_dtypes.bfloat16


def _build_bass():
    nc = bass.Bass("TRN2", target_bir_lowering=False, debug=False,
                   num_devices=N_CORES)

    # DRAM I/O (per core)
    Wp_d = nc.dram_tensor("Wp", [128, NT * CO], BF16, kind="ExternalInput")
    uT_d = nc.dram_tensor("uT", [128, NT * B_LOC], BF16, kind="ExternalInput")
    ub_d = nc.dram_tensor("ub", [B_LOC, RI], BF16, kind="ExternalInput")
    E_d = nc.dram_tensor("E", [128, 16], BF16, kind="ExternalInput")
    R8_d = nc.dram_tensor("R8", [16, 128], BF16, kind="ExternalInput")
    o16_d = nc.dram_tensor("o16", [16, 1], BF16, kind="ExternalInput")
    o1x32_d = nc.dram_tensor("o1x32", [1, 32], F32, kind="ExternalInput")
    y_d = nc.dram_tensor("y", [B_LOC, CO], F32, kind="ExternalOutput")

    rg = [list(range(N_CORES))]

    with tile.TileContext(nc) as tc:
        with (
            tc.tile_pool(name="big", bufs=1) as big,
            tc.tile_pool(name="small", bufs=1) as small,
            tc.tile_pool(name="work", bufs=2) as work,
            tc.tile_pool(name="psum", bufs=1, space="PSUM") as psum,
            tc.tile_pool(name="tpsum", bufs=3, space="PSUM") as tpsum,
            tc.tile_pool(name="dram", bufs=8, space="DRAM") as dram,
        ):
            # ---------------- persistent SBUF ----------------
            W_sb = big.tile([128, NT, O, C], BF16, tag="W")
            Wc_sb = big.tile([128, NT, O, C], BF16, tag="Wc")
            P_sb = big.tile([128, NT, O, C], BF16, tag="P")
            uT_sb = big.tile([128, NT, B_LOC], BF16, tag="uT")
            ub_sb = big.tile([B_LOC, RI], BF16, tag="ub")
            Q_sb = big.tile([128, NT * C], BF16, tag="Q")
            H8_sb = big.tile([128, NT, 8, C], BF16, tag="H8")
            H4_sb = big.tile([128, NT, 4, C], BF16, tag="H4")
            H2_sb = big.tile([128, NT, 2, C], BF16, tag="H2")

            E_sb = small.tile([128, 16], BF16, tag="E")
            R8_sb = small.tile([16, 128], BF16, tag="R8")
            o16_sb = small.tile([16, 1], BF16, tag="o16")
            o1x32_sb = small.tile([1, 32], F32, tag="o1x32")
            b_sb = small.tile([16, NT * C], F32, tag="b")    # routing logits
            s_sb = small.tile([B_LOC, CO], F32, tag="s")     # squash scratch
            xn_sb = small.tile([B_LOC, CO], F32, tag="xn")
            ab_sb = small.tile([B_LOC, CO], F32, tag="ab")
            den_sb = small.tile([B_LOC, CO], F32, tag="den")
            num_sb = small.tile([B_LOC, CO], F32, tag="num")
            v_sb = small.tile([B_LOC, CO], F32, tag="v")

            # constants
            nc.sync.dma_start(out=E_sb[:], in_=E_d[:])
            nc.sync.dma_start(out=R8_sb[:], in_=R8_d[:])
            nc.sync.dma_start(out=o16_sb[:], in_=o16_d[:])
            nc.sync.dma_start(out=o1x32_sb[:], in_=o1x32_d[:])
            nc.vector.memset(b_sb[:], 0.0)

            # bulk loads, chunked so iter-1 matmuls can start early
            Wp_v = Wp_d[:].rearrange("p (t f) -> p t f", t=NT)
            uT_v = uT_d[:].rearrange("p (t f) -> p t f", t=NT)
            for ch in range(NT // NCHUNK):
                sl = slice(ch * NCHUNK, (ch + 1) * NCHUNK)
                nc.sync.dma_start(
                    out=W_sb[:, sl, :, :],
                    in_=Wp_v[:, sl, :].rearrange("p t (o c) -> p t o c", o=O))
                nc.sync.dma_start(out=uT_sb[:, sl, :], in_=uT_v[:, sl, :])
            nc.sync.dma_start(out=ub_sb[:], in_=ub_d[:])

            def softmax_and_scale(it):
                """b -> exp(b) -> Wexp; 1/sum(exp) is applied to s columns
                later (softmax denominator commutes through the s-matmul),
                keeping the denominator chain off the critical path."""
                exp_bf = work.tile([16, NT * C], BF16, tag="exp")
                nc.scalar.activation(exp_bf[:], b_sb[:],
                                     mybir.ActivationFunctionType.Exp)
                # denominator branch (consumed only at squash time):
                den_ps = psum.tile([1, NT * C], F32, tag="A1")
                nc.tensor.matmul(den_ps[:, 0:512], o16_sb[:], exp_bf[:, 0:512])
                nc.tensor.matmul(den_ps[:, 512:720], o16_sb[:],
                                 exp_bf[:, 512:720])
                den_c = work.tile([1, C], F32, tag="denc")
                nc.vector.reduce_sum(
                    den_c[:],
                    den_ps[:].rearrange("p (t c) -> p c t", t=NT),
                    axis=mybir.AxisListType.X)
                rden = work.tile([1, C], F32, tag="rden")
                nc.vector.reciprocal(rden[:], den_c[:])
                rden32_ps = psum.tile([B_LOC, C], F32, tag="A1")
                nc.tensor.matmul(rden32_ps[:], o1x32_sb[:], rden[:])
                rden32 = work.tile([B_LOC, C], F32, tag="rden32")
                nc.scalar.copy(rden32[:], rden32_ps[:])
                # replicate exp over i: crep[p=(r8+i), (t,c)] = exp[r, (t,c)]
                crep_ps = psum.tile([128, NT * C], F32, tag="A2")
                nc.tensor.matmul(crep_ps[:, 0:512], R8_sb[:], exp_bf[:, 0:512])
                nc.tensor.matmul(crep_ps[:, 512:720], R8_sb[:],
                                 exp_bf[:, 512:720])
                crep_bf = work.tile([128, NT * C], BF16, tag="crepbf")
                nc.scalar.copy(crep_bf[:, 0:240], crep_ps[:, 0:240])
                nc.scalar.copy(crep_bf[:, 240:480], crep_ps[:, 240:480])
                nc.scalar.copy(crep_bf[:, 480:720], crep_ps[:, 480:720])
                # Wc = W * c (o broadcast in the middle free dim keeps the
                # innermost c unit-stride -> DVE 2x mode). 3 chunks so the
                # s-chain can start after the first and GpSimd helps.
                crep_v = crep_bf[:].rearrange("p (t c) -> p t c", t=NT)
                for ci, (t0, t1, eng) in enumerate(
                        [(0, 24, nc.vector), (24, 48, nc.vector),
                         (48, 72, nc.vector)]):
                    eng.tensor_mul(
                        Wc_sb[:, t0:t1, :, :],
                        W_sb[:, t0:t1, :, :],
                        crep_v[:, t0:t1, :].unsqueeze(2)
                        .broadcast_to([128, t1 - t0, O, C]))
                return rden32

            def s_chain(it):
                """s[b, co] = uT^T @ (Wc or W), K = 9216 chained."""
                rhs = W_sb if it == 0 else Wc_sb
                s_ps = psum.tile([B_LOC, CO], F32, tag="S0")
                for t in range(NT):
                    nc.tensor.matmul(s_ps[:], uT_sb[:, t, :],
                                     rhs[:, t, :, :],
                                     start=(t == 0), stop=(t == NT - 1))
                return s_ps

            def squash(s_ps, it, rden32=None):
                """v = s*|s|/(1+s^2) elementwise on [b, (o,c)] (fp32).
                The softmax denominator (or the uniform 1/R on iter 0) is
                applied to s columns here."""
                if it == 0:
                    nc.scalar.mul(s_sb[:], s_ps[:], 1.0 / R)
                else:
                    nc.vector.tensor_mul(
                        s_sb[:].rearrange("b (o c) -> b o c", o=O),
                        s_ps[:].rearrange("b (o c) -> b o c", o=O),
                        rden32[:].unsqueeze(1).broadcast_to([B_LOC, O, C]))
                nc.vector.tensor_mul(xn_sb[:], s_sb[:], s_sb[:])
                nc.scalar.activation(ab_sb[:], s_sb[:],
                                     mybir.ActivationFunctionType.Abs)
                nc.scalar.add(den_sb[:], xn_sb[:], 1.0)
                nc.vector.reciprocal(den_sb[:], den_sb[:])
                nc.vector.tensor_mul(num_sb[:], s_sb[:], ab_sb[:])
                nc.vector.tensor_mul(v_sb[:], num_sb[:], den_sb[:])

            def a_phase(v_bf):
                """T = u_b^T @ v per ri-tile (3 tiles per PSUM bank);
                P = W*T batched per bank; Q = sum_o P; a = E^T @ Q."""
                NG = NT // TB
                for g in range(NG):
                    T_ps = tpsum.tile([128, TB * CO], F32, tag="T")
                    for j in range(TB):
                        t = g * TB + j
                        nc.tensor.matmul(
                            T_ps[:, j * CO:(j + 1) * CO],
                            ub_sb[:, t * 128:(t + 1) * 128], v_bf[:])
                    if g % 24 not in (0, 3, 7, 10, 14, 17, 21):
                        # ACT copies the bank to SBUF so the multiply runs
                        # unit-stride bf16 at 2x on DVE
                        T_cp = work.tile([128, TB * CO], BF16, tag="tcp")
                        nc.scalar.copy(T_cp[:], T_ps[:])
                        nc.vector.tensor_mul(
                            P_sb[:, g * TB:(g + 1) * TB, :, :],
                            W_sb[:, g * TB:(g + 1) * TB, :, :],
                            T_cp[:].rearrange("p (j o c) -> p j o c",
                                              j=TB, o=O))
                    else:
                        nc.vector.tensor_mul(
                            P_sb[:, g * TB:(g + 1) * TB, :, :],
                            W_sb[:, g * TB:(g + 1) * TB, :, :],
                            T_ps[:].rearrange("p (j o c) -> p j o c",
                                              j=TB, o=O))
                    if g == NG // 2 - 1:
                        # first-half o-sums overlap the second half's T/P
                        nc.vector.tensor_add(
                            H8_sb[:, 0:NT // 2, :, :],
                            P_sb[:, 0:NT // 2, 0:8, :],
                            P_sb[:, 0:NT // 2, 8:16, :])
                        nc.vector.tensor_add(
                            H4_sb[:, 0:NT // 2, :, :],
                            H8_sb[:, 0:NT // 2, 0:4, :],
                            H8_sb[:, 0:NT // 2, 4:8, :])
                # sum over o: pairwise halving keeps unit-stride c-runs (2x)
                nc.vector.tensor_add(H8_sb[:, NT // 2:NT, :, :],
                                     P_sb[:, NT // 2:NT, 0:8, :],
                                     P_sb[:, NT // 2:NT, 8:16, :])
                nc.vector.tensor_add(H4_sb[:, NT // 2:NT, :, :],
                                     H8_sb[:, NT // 2:NT, 0:4, :],
                                     H8_sb[:, NT // 2:NT, 4:8, :])
                nc.vector.tensor_add(H2_sb[:], H4_sb[:, :, 0:2, :],
                                     H4_sb[:, :, 2:4, :])
                nc.vector.tensor_add(
                    Q_sb[:].rearrange("p (t c) -> p t c", t=NT),
                    H2_sb[:, :, 0, :], H2_sb[:, :, 1, :])
                a_ps = psum.tile([16, NT * C], F32, tag="A1")
                nc.tensor.matmul(a_ps[:, 0:512], E_sb[:], Q_sb[:, 0:512])
                nc.tensor.matmul(a_ps[:, 512:720], E_sb[:], Q_sb[:, 512:720])
                return a_ps

            for it in range(NUM_ITERS):
                rden32 = softmax_and_scale(it) if it > 0 else None
                s_ps = s_chain(it)
                squash(s_ps, it, rden32)
                if it < NUM_ITERS - 1:
                    v_bf = work.tile([B_LOC, CO], BF16, tag="vbf")
                    nc.vector.tensor_copy(v_bf[:], v_sb[:])
                    a_ps = a_phase(v_bf)
                    a_sb = work.tile([16, NT * C], BF16, tag="asb")
                    nc.scalar.copy(a_sb[:], a_ps[:])
                    ar_in = dram.tile([16, NT * C], BF16, tag="arin")
                    ar_out = dram.tile([16, NT * C], BF16, tag="arout")
                    nc.sync.dma_start(out=ar_in[:], in_=a_sb[:])
                    nc.gpsimd.collective_compute(
                        "AllReduce", mybir.AluOpType.add,
                        replica_groups=rg,
                        ins=[ar_in.opt()], outs=[ar_out.opt()])
                    a_red = work.tile([16, NT * C], BF16, tag="ared")
                    nc.sync.dma_start(out=a_red[:], in_=ar_out[:])
                    nc.vector.tensor_add(b_sb[:], b_sb[:], a_red[:])
                else:
                    nc.sync.dma_start(out=y_d[:], in_=v_sb[:])

    _legalize_sync_waits(nc)
    return nc


def _host_prep(u, W):
    """Build per-core input maps from full inputs."""
    u = np.ascontiguousarray(np.asarray(u, dtype=np.float32))
    W = np.ascontiguousarray(np.asarray(W, dtype=np.float32))

    W_perm = W[0].transpose(0, 3, 2, 1).reshape(RI, CO)          # [ri, (o,c)]
    Wp = np.ascontiguousarray(
        W_perm.reshape(NT, 128, CO).transpose(1, 0, 2).reshape(128, NT * CO)
    ).astype(bfnp)

    E = np.zeros((128, 16), np.float32)
    E[np.arange(128), np.arange(128) // 8] = 1.0 / B
    E = E.astype(bfnp)
    R8 = np.zeros((16, 128), np.float32)
    R8[np.arange(128) // 8, np.arange(128)] = 1.0
    R8 = R8.astype(bfnp)
    o16 = np.ones((16, 1), np.float32).astype(bfnp)
    o1x32 = np.ones((1, 32), np.float32)

    in_maps = []
    for c in range(N_CORES):
        u_loc = u[c * B_LOC:(c + 1) * B_LOC]                     # [32, R, I]
        u_flat = u_loc.reshape(B_LOC, RI)
        uT = np.ascontiguousarray(
            u_flat.T.reshape(NT, 128, B_LOC).transpose(1, 0, 2)
            .reshape(128, NT * B_LOC)).astype(bfnp)
        ub = np.ascontiguousarray(u_flat).astype(bfnp)
        in_maps.append({
            "Wp": Wp, "uT": uT, "ub": ub, "E": E, "R8": R8,
            "o16": o16, "o1x32": o1x32,
        })
    return in_maps


_cached = {}


def _get_nc():
    if "nc" not in _cached:
        _cached["nc"] = _build_bass()
    return _cached["nc"]


def kernel(u, W, _return_timing=False):
    nc = _get_nc()
    in_maps = _host_prep(u, W)
    res = run_bass_kernel_spmd(
        nc, in_maps, list(range(N_CORES)), trace=_return_timing)
    outs = [res.results[i]["y"].reshape(B_LOC, O, C).transpose(0, 2, 1)
            .reshape(B_LOC, C, O, 1) for i in range(N_CORES)]
    full = np.concatenate(outs, axis=0).astype(np.float32)
    if _return_timing:
        return full, res.exec_time_ns
    return full


# revision 14
# speedup vs baseline: 1.0234x; 1.0234x over previous
"""Trainium2 Bass kernel for nn_DigitCapsuleLayer (dynamic-routing capsule layer).

Strategy
--------
Data-parallel over batch (8 cores x 32). u_hat (189 MB full) is never
materialized: every consumer is re-expressed as a matmul against the
(ri)=(r,i)-flattened operands, so per-core HBM traffic is just u + W.

Per routing iteration (3 total; the 3rd iteration's b-update is dead code
and skipped, so only 2 AllReduces):
  c_ij   = softmax(b) over R            (fp32; PE partition-sums + DVE)
  Wc     = W * c_ij                     (DVE + GpSimd, bf16)
  s      = uT^T @ Wc                    (PE, K=9216 chain, out [b, co])
  v      = squash(s)                    (ACT+DVE, fp32)
  T      = u_b^T @ v                    (PE per ri-tile, fp32 PSUM)
  a      = E^T @ (reduce_o(W*T))        (DVE mul+reduce, PE group-reduce)
  b     += AllReduce_batchmean(a)       (23 KB bf16 collective)

All matmul operands are bf16 (fp32 PSUM accumulation); the routing state
b/c/softmax/squash stays fp32. Measured end-to-end relative error ~3e-3.
"""

import sys

sys.path.insert(0, "/opt/trn_rl_repo")

import numpy as np
import ml_dtypes

import concourse.bass as bass
import concourse.tile as tile
from concourse import mybir
from concourse.bass_utils import run_bass_kernel_spmd
from concourse.vector_clock import ScopedClock

# ----------------------------------------------------------------------------
# Walrus workarounds: this image's walrus rejects any instruction carrying
# more than one sync wait. Split Tile's tail-drain waits and any other
# multi-wait instruction into single-wait NOPs on the same engine.
# ----------------------------------------------------------------------------

_uid = [0]


def _patched_drain_and_barrier(self, tick_clock, wait_clock):
    nc = self.nc
    probe = nc.sync.nop(nofuse=True, hint="tail_drain_waits")
    wait_clock.add_sem_waits(probe.ins, ScopedClock({None: tick_clock.global_clock}))
    si = probe.ins.sync_info
    waits = list(si.on_wait) if si is not None else []
    probe.ins.sync_info = mybir.SyncInfo(on_wait=waits[:1], on_update=[])
    for w in waits[1:]:
        n = nc.sync.nop(nofuse=True, hint="tail_drain_waits")
        n.ins.sync_info = mybir.SyncInfo(on_wait=[w], on_update=[])
    nc.sync.drain()
    nc.all_engine_barrier(sem_only=True)
    assert self.sems is not None
    popped = nc._tile_sem_poison_stack.pop()
    assert popped is self._sem_poison
    nc.clear_and_free_semaphores(list(self.sems.allocated().values()))


tile.TileContext._drain_and_barrier = _patched_drain_and_barrier


def _legalize_sync_waits(nc):
    for fn in nc.m.functions:
        for bb in fn.blocks:
            insts = bb.instructions
            i = 0
            while i < len(insts):
                inst = insts[i]
                si = getattr(inst, "sync_info", None)
                waits = list(si.on_wait) if si is not None else []
                if len(waits) > 1:
                    for w in waits[:-1]:
                        _uid[0] += 1
                        nop = mybir.InstNoOp(
                            name=f"I-waitsplit-{_uid[0]}", ins=[], outs=[]
                        )
                        nop.engine = inst.engine
                        nop.sync_info = mybir.SyncInfo(on_wait=[w], on_update=[])
                        insts.insert(i, nop)
                        i += 1
                    inst.sync_info = mybir.SyncInfo(
                        on_wait=[waits[-1]], on_update=list(si.on_update)
                    )
                i += 1


# ----------------------------------------------------------------------------
# Problem constants (hardcoded per contest contract)
# ----------------------------------------------------------------------------

B, R, C, O, I = 256, 1152, 10, 16, 8
NUM_ITERS = 3
N_CORES = 8
B_LOC = B // N_CORES          # 32
RI = R * I                    # 9216
CO = C * O                    # 160
NT = RI // 128                # 72 ri-tiles
NCHUNK = 8                    # ri-tiles per load/scale chunk
TB = 3                        # T-matmul tiles packed per PSUM bank
F32 = mybir.dt.float32
BF16 = mybir.dt.bfloat16
bfnp = ml_dtypes.bfloat16


def _build_bass():
    nc = bass.Bass("TRN2", target_bir_lowering=False, debug=False,
                   num_devices=N_CORES)

    # DRAM I/O (per core)
    Wp_d = nc.dram_tensor("Wp", [128, NT * CO], BF16, kind="ExternalInput")
    uT_d = nc.dram_tensor("uT", [128, NT * B_LOC], BF16, kind="ExternalInput")
    ub_d = nc.dram_tensor("ub", [B_LOC, RI], BF16, kind="ExternalInput")
    E_d = nc.dram_tensor("E", [128, 16], BF16, kind="ExternalInput")
    R8_d = nc.dram_tensor("R8", [16, 128], BF16, kind="ExternalInput")
    o16_d = nc.dram_tensor("o16", [16, 1], BF16, kind="ExternalInput")
    o1x32_d = nc.dram_tensor("o1x32", [1, 32], F32, kind="ExternalInput")
    y_d = nc.dram_tensor("y", [B_LOC, CO], F32, kind="ExternalOutput")

    rg = [list(range(N_CORES))]

    with tile.TileContext(nc) as tc:
        with (
            tc.tile_pool(name="big", bufs=1) as big,
            tc.tile_pool(name="small", bufs=1) as small,
            tc.tile_pool(name="work", bufs=2) as work,
            tc.tile_pool(name="psum", bufs=1, space="PSUM") as psum,
            tc.tile_pool(name="tpsum", bufs=3, space="PSUM") as tpsum,
            tc.tile_pool(name="dram", bufs=8, space="DRAM") as dram,
        ):
            # ---------------- persistent SBUF ----------------
            W_sb = big.tile([128, NT, O, C], BF16, tag="W")
            Wc_sb = big.tile([128, NT, O, C], BF16, tag="Wc")
            P_sb = big.tile([128, NT, O, C], BF16, tag="P")
            uT_sb = big.tile([128, NT, B_LOC], BF16, tag="uT")
            ub_sb = big.tile([B_LOC, RI], BF16, tag="ub")
            Q_sb = big.tile([128, NT * C], BF16, tag="Q")
            H8_sb = big.tile([128, NT, 8, C], BF16, tag="H8")
            H4_sb = big.tile([128, NT, 4, C], BF16, tag="H4")
            H2_sb = big.tile([128, NT, 2, C], BF16, tag="H2")

            E_sb = small.tile([128, 16], BF16, tag="E")
            R8_sb = small.tile([16, 128], BF16, tag="R8")
            o16_sb = small.tile([16, 1], BF16, tag="o16")
            o1x32_sb = small.tile([1, 32], F32, tag="o1x32")
            b_sb = small.tile([16, NT * C], F32, tag="b")    # routing logits
            s_sb = small.tile([B_LOC, CO], F32, tag="s")     # squash scratch
            xn_sb = small.tile([B_LOC, CO], F32, tag="xn")
            ab_sb = small.tile([B_LOC, CO], F32, tag="ab")
            den_sb = small.tile([B_LOC, CO], F32, tag="den")
            num_sb = small.tile([B_LOC, CO], F32, tag="num")
            v_sb = small.tile([B_LOC, CO], F32, tag="v")

            # constants
            nc.sync.dma_start(out=E_sb[:], in_=E_d[:])
            nc.sync.dma_start(out=R8_sb[:], in_=R8_d[:])
            nc.sync.dma_start(out=o16_sb[:], in_=o16_d[:])
            nc.sync.dma_start(out=o1x32_sb[:], in_=o1x32_d[:])
            nc.vector.memset(b_sb[:], 0.0)

            # bulk loads, chunked so iter-1 matmuls can start early
            Wp_v = Wp_d[:].rearrange("p (t f) -> p t f", t=NT)
            uT_v = uT_d[:].rearrange("p (t f) -> p t f", t=NT)
            for ch in range(NT // NCHUNK):
                sl = slice(ch * NCHUNK, (ch + 1) * NCHUNK)
                nc.sync.dma_start(
                    out=W_sb[:, sl, :, :],
                    in_=Wp_v[:, sl, :].rearrange("p t (o c) -> p t o c", o=O))
                nc.sync.dma_start(out=uT_sb[:, sl, :], in_=uT_v[:, sl, :])
            nc.sync.dma_start(out=ub_sb[:], in_=ub_d[:])

            def softmax_and_scale(it):
                """b -> exp(b) -> Wexp; 1/sum(exp) is applied to s columns
                later (softmax denominator commutes through the s-matmul),
                keeping the denominator chain off the critical path."""
                exp_bf = work.tile([16, NT * C], BF16, tag="exp")
                nc.scalar.activation(exp_bf[:], b_sb[:],
                                     mybir.ActivationFunctionType.Exp)
                # denominator branch (consumed only at squash time):
                den_ps = psum.tile([1, NT * C], F32, tag="A1")
                nc.tensor.matmul(den_ps[:, 0:512], o16_sb[:], exp_bf[:, 0:512])
                nc.tensor.matmul(den_ps[:, 512:720], o16_sb[:],
                                 exp_bf[:, 512:720])
                den_c = work.tile([1, C], F32, tag="denc")
                nc.vector.reduce_sum(
                    den_c[:],
                    den_ps[:].rearrange("p (t c) -> p c t", t=NT),
                    axis=mybir.AxisListType.X)
                rden = work.tile([1, C], F32, tag="rden")
                nc.vector.reciprocal(rden[:], den_c[:])
                rden32_ps = psum.tile([B_LOC, C], F32, tag="A1")
                nc.tensor.matmul(rden32_ps[:], o1x32_sb[:], rden[:])
                rden32 = work.tile([B_LOC, C], F32, tag="rden32")
                nc.scalar.copy(rden32[:], rden32_ps[:])
                # replicate exp over i: crep[p=(r8+i), (t,c)] = exp[r, (t,c)]
                crep_ps = psum.tile([128, NT * C], F32, tag="A2")
                nc.tensor.matmul(crep_ps[:, 0:512], R8_sb[:], exp_bf[:, 0:512])
                nc.tensor.matmul(crep_ps[:, 512:720], R8_sb[:],
                                 exp_bf[:, 512:720])
                crep_bf = work.tile([128, NT * C], BF16, tag="crepbf")
                nc.scalar.copy(crep_bf[:, 0:240], crep_ps[:, 0:240])
                nc.scalar.copy(crep_bf[:, 240:480], crep_ps[:, 240:480])
                nc.scalar.copy(crep_bf[:, 480:720], crep_ps[:, 480:720])
                # Wc = W * c (o broadcast in the middle free dim keeps the
                # innermost c unit-stride -> DVE 2x mode). 3 chunks so the
                # s-chain can start after the first and GpSimd helps.
                crep_v = crep_bf[:].rearrange("p (t c) -> p t c", t=NT)
                for ci, (t0, t1, eng) in enumerate(
                        [(0, 24, nc.vector), (24, 48, nc.vector),
                         (48, 72, nc.vector)]):
                    eng.tensor_mul(
                        Wc_sb[:, t0:t1, :, :],
                        W_sb[:, t0:t1, :, :],
                        crep_v[:, t0:t1, :].unsqueeze(2)
                        .broadcast_to([128, t1 - t0, O, C]))
                return rden32

            def s_chain(it):
                """s[b, co] = uT^T @ (Wc or W), K = 9216 chained."""
                rhs = W_sb if it == 0 else Wc_sb
                s_ps = psum.tile([B_LOC, CO], F32, tag="S0")
                for t in range(NT):
                    nc.tensor.matmul(s_ps[:], uT_sb[:, t, :],
                                     rhs[:, t, :, :],
                                     start=(t == 0), stop=(t == NT - 1))
                return s_ps

            def squash(s_ps, it, rden32=None):
                """v = s*|s|/(1+s^2) elementwise on [b, (o,c)] (fp32).
                The softmax denominator (or the uniform 1/R on iter 0) is
                applied to s columns here."""
                if it == 0:
                    nc.scalar.mul(s_sb[:], s_ps[:], 1.0 / R)
                else:
                    nc.vector.tensor_mul(
                        s_sb[:].rearrange("b (o c) -> b o c", o=O),
                        s_ps[:].rearrange("b (o c) -> b o c", o=O),
                        rden32[:].unsqueeze(1).broadcast_to([B_LOC, O, C]))
                nc.vector.tensor_mul(xn_sb[:], s_sb[:], s_sb[:])
                nc.scalar.activation(ab_sb[:], s_sb[:],
                                     mybir.ActivationFunctionType.Abs)
                nc.scalar.add(den_sb[:], xn_sb[:], 1.0)
                nc.vector.reciprocal(den_sb[:], den_sb[:])
                nc.vector.tensor_mul(num_sb[:], s_sb[:], ab_sb[:])
                if it < NUM_ITERS - 1:
                    v_bf = work.tile([B_LOC, CO], BF16, tag="vbf")
                    nc.vector.tensor_mul(v_bf[:], num_sb[:], den_sb[:])
                    return v_bf
                nc.vector.tensor_mul(v_sb[:], num_sb[:], den_sb[:])
                return None

            def a_phase(v_bf):
                """T = u_b^T @ v per ri-tile (3 tiles per PSUM bank);
                P = W*T batched per bank; Q = sum_o P; a = E^T @ Q."""
                NG = NT // TB
                for g in range(NG):
                    T_ps = tpsum.tile([128, TB * CO], F32, tag="T")
                    for j in range(TB):
                        t = g * TB + j
                        nc.tensor.matmul(
                            T_ps[:, j * CO:(j + 1) * CO],
                            ub_sb[:, t * 128:(t + 1) * 128], v_bf[:])
                    if g % 6 > 0:
                        # ACT copies the bank to SBUF so the multiply runs
                        # unit-stride bf16 at 2x on DVE
                        T_cp = work.tile([128, TB * CO], BF16, tag="tcp")
                        nc.scalar.copy(T_cp[:], T_ps[:])
                        nc.vector.tensor_mul(
                            P_sb[:, g * TB:(g + 1) * TB, :, :],
                            W_sb[:, g * TB:(g + 1) * TB, :, :],
                            T_cp[:].rearrange("p (j o c) -> p j o c",
                                              j=TB, o=O))
                    else:
                        nc.vector.tensor_mul(
                            P_sb[:, g * TB:(g + 1) * TB, :, :],
                            W_sb[:, g * TB:(g + 1) * TB, :, :],
                            T_ps[:].rearrange("p (j o c) -> p j o c",
                                              j=TB, o=O))
                    if g == NG // 2 - 1:
                        # first-half o-sums overlap the second half's T/P
                        nc.vector.tensor_add(
                            H8_sb[:, 0:NT // 2, :, :],
                            P_sb[:, 0:NT // 2, 0:8, :],
                            P_sb[:, 0:NT // 2, 8:16, :])
                        nc.vector.tensor_add(
                            H4_sb[:, 0:NT // 2, :, :],
                            H8_sb[:, 0:NT // 2, 0:4, :],
                            H8_sb[:, 0:NT // 2, 4:8, :])
                # sum over o: pairwise halving keeps unit-stride c-runs (2x)
                nc.vector.tensor_add(H8_sb[:, NT // 2:NT, :, :],
                                     P_sb[:, NT // 2:NT, 0:8, :],
                                     P_sb[:, NT // 2:NT, 8:16, :])
                nc.vector.tensor_add(H4_sb[:, NT // 2:NT, :, :],
                                     H8_sb[:, NT // 2:NT, 0:4, :],
                                     H8_sb[:, NT // 2:NT, 4:8, :])
                nc.vector.tensor_add(H2_sb[:], H4_sb[:, :, 0:2, :],
                                     H4_sb[:, :, 2:4, :])
                nc.vector.tensor_add(
                    Q_sb[:].rearrange("p (t c) -> p t c", t=NT),
                    H2_sb[:, :, 0, :], H2_sb[:, :, 1, :])
                a_ps = psum.tile([16, NT * C], F32, tag="A1")
                nc.tensor.matmul(a_ps[:, 0:512], E_sb[:], Q_sb[:, 0:512])
                nc.tensor.matmul(a_ps[:, 512:720], E_sb[:], Q_sb[:, 512:720])
                return a_ps

            for it in range(NUM_ITERS):
                rden32 = softmax_and_scale(it) if it > 0 else None
                s_ps = s_chain(it)
                v_bf = squash(s_ps, it, rden32)
                if it < NUM_ITERS - 1:
                    a_ps = a_phase(v_bf)
                    a_sb = work.tile([16, NT * C], BF16, tag="asb")
                    nc.scalar.copy(a_sb[:], a_ps[:])
                    ar_in = dram.tile([16, NT * C], BF16, tag="arin")
                    ar_out = dram.tile([16, NT * C], BF16, tag="arout")
                    nc.sync.dma_start(out=ar_in[:], in_=a_sb[:])
                    nc.gpsimd.collective_compute(
                        "AllReduce", mybir.AluOpType.add,
                        replica_groups=rg,
                        ins=[ar_in.opt()], outs=[ar_out.opt()])
                    a_red = work.tile([16, NT * C], BF16, tag="ared")
                    nc.sync.dma_start(out=a_red[:], in_=ar_out[:])
                    nc.vector.tensor_add(b_sb[:], b_sb[:], a_red[:])
                else:
                    nc.sync.dma_start(out=y_d[:], in_=v_sb[:])

    _legalize_sync_waits(nc)
    return nc


def _host_prep(u, W):
    """Build per-core input maps from full inputs."""
    u = np.ascontiguousarray(np.asarray(u, dtype=np.float32))
    W = np.ascontiguousarray(np.asarray(W, dtype=np.float32))

    W_perm = W[0].transpose(0, 3, 2, 1).reshape(RI, CO)          # [ri, (o,c)]
    Wp = np.ascontiguousarray(
        W_perm.reshape(NT, 128, CO).transpose(1, 0, 2).reshape(128, NT * CO)
    ).astype(bfnp)

    E = np.zeros((128, 16), np.float32)
    E[np.arange(128), np.arange(128) // 8] = 1.0 / B
    E = E.astype(bfnp)
    R8 = np.zeros((16, 128), np.float32)
    R8[np.arange(128) // 8, np.arange(128)] = 1.0
    R8 = R8.astype(bfnp)
    o16 = np.ones((16, 1), np.float32).astype(bfnp)
    o1x32 = np.ones((1, 32), np.float32)

    in_maps = []
    for c in range(N_CORES):
        u_loc = u[c * B_LOC:(c + 1) * B_LOC]                     # [32, R, I]
        u_flat = u_loc.reshape(B_LOC, RI)
        uT = np.ascontiguousarray(
            u_flat.T.reshape(NT, 128, B_LOC).transpose(1, 0, 2)
            .reshape(128, NT * B_LOC)).astype(bfnp)
        ub = np.ascontiguousarray(u_flat).astype(bfnp)
        in_maps.append({
            "Wp": Wp, "uT": uT, "ub": ub, "E": E, "R8": R8,
            "o16": o16, "o1x32": o1x32,
        })
    return in_maps


_cached = {}


def _get_nc():
    if "nc" not in _cached:
        _cached["nc"] = _build_bass()
    return _cached["nc"]


def kernel(u, W, _return_timing=False):
    nc = _get_nc()
    in_maps = _host_prep(u, W)
    res = run_bass_kernel_spmd(
        nc, in_maps, list(range(N_CORES)), trace=_return_timing)
    outs = [res.results[i]["y"].reshape(B_LOC, O, C).transpose(0, 2, 1)
            .reshape(B_LOC, C, O, 1) for i in range(N_CORES)]
    full = np.concatenate(outs, axis=0).astype(np.float32)
    if _return_timing:
        return full, res.exec_time_ns
    return full


# revision 16
# speedup vs baseline: 1.0322x; 1.0086x over previous
"""Trainium2 Bass kernel for nn_DigitCapsuleLayer (dynamic-routing capsule layer).

Strategy
--------
Data-parallel over batch (8 cores x 32). u_hat (189 MB full) is never
materialized: every consumer is re-expressed as a matmul against the
(ri)=(r,i)-flattened operands, so per-core HBM traffic is just u + W.

Per routing iteration (3 total; the 3rd iteration's b-update is dead code
and skipped, so only 2 AllReduces):
  c_ij   = softmax(b) over R            (fp32; PE partition-sums + DVE)
  Wc     = W * c_ij                     (DVE + GpSimd, bf16)
  s      = uT^T @ Wc                    (PE, K=9216 chain, out [b, co])
  v      = squash(s)                    (ACT+DVE, fp32)
  T      = u_b^T @ v                    (PE per ri-tile, fp32 PSUM)
  a      = E^T @ (reduce_o(W*T))        (DVE mul+reduce, PE group-reduce)
  b     += AllReduce_batchmean(a)       (23 KB bf16 collective)

All matmul operands are bf16 (fp32 PSUM accumulation); the routing state
b/c/softmax/squash stays fp32. Measured end-to-end relative error ~3e-3.
"""

import sys

sys.path.insert(0, "/opt/trn_rl_repo")

import numpy as np
import ml_dtypes

import concourse.bass as bass
import concourse.tile as tile
from concourse import mybir
from concourse.bass_utils import run_bass_kernel_spmd
from concourse.vector_clock import ScopedClock

# ----------------------------------------------------------------------------
# Walrus workarounds: this image's walrus rejects any instruction carrying
# more than one sync wait. Split Tile's tail-drain waits and any other
# multi-wait instruction into single-wait NOPs on the same engine.
# ----------------------------------------------------------------------------

_uid = [0]


def _patched_drain_and_barrier(self, tick_clock, wait_clock):
    nc = self.nc
    probe = nc.sync.nop(nofuse=True, hint="tail_drain_waits")
    wait_clock.add_sem_waits(probe.ins, ScopedClock({None: tick_clock.global_clock}))
    si = probe.ins.sync_info
    waits = list(si.on_wait) if si is not None else []
    probe.ins.sync_info = mybir.SyncInfo(on_wait=waits[:1], on_update=[])
    for w in waits[1:]:
        n = nc.sync.nop(nofuse=True, hint="tail_drain_waits")
        n.ins.sync_info = mybir.SyncInfo(on_wait=[w], on_update=[])
    nc.sync.drain()
    nc.all_engine_barrier(sem_only=True)
    assert self.sems is not None
    popped = nc._tile_sem_poison_stack.pop()
    assert popped is self._sem_poison
    nc.clear_and_free_semaphores(list(self.sems.allocated().values()))


tile.TileContext._drain_and_barrier = _patched_drain_and_barrier


def _legalize_sync_waits(nc):
    for fn in nc.m.functions:
        for bb in fn.blocks:
            insts = bb.instructions
            i = 0
            while i < len(insts):
                inst = insts[i]
                si = getattr(inst, "sync_info", None)
                waits = list(si.on_wait) if si is not None else []
                if len(waits) > 1:
                    for w in waits[:-1]:
                        _uid[0] += 1
                        nop = mybir.InstNoOp(
                            name=f"I-waitsplit-{_uid[0]}", ins=[], outs=[]
                        )
                        nop.engine = inst.engine
                        nop.sync_info = mybir.SyncInfo(on_wait=[w], on_update=[])
                        insts.insert(i, nop)
                        i += 1
                    inst.sync_info = mybir.SyncInfo(
                        on_wait=[waits[-1]], on_update=list(si.on_update)
                    )
                i += 1


# ----------------------------------------------------------------------------
# Problem constants (hardcoded per contest contract)
# ----------------------------------------------------------------------------

B, R, C, O, I = 256, 1152, 10, 16, 8
NUM_ITERS = 3
N_CORES = 8
B_LOC = B // N_CORES          # 32
RI = R * I                    # 9216
CO = C * O                    # 160
NT = RI // 128                # 72 ri-tiles
NCHUNK = 8                    # ri-tiles per load/scale chunk
TB = 3                        # T-matmul tiles packed per PSUM bank
F32 = mybir.dt.float32
BF16 = mybir.dt.bfloat16
bfnp = ml_dtypes.bfloat16


def _build_bass():
    nc = bass.Bass("TRN2", target_bir_lowering=False, debug=False,
                   num_devices=N_CORES)

    # DRAM I/O (per core)
    Wp_d = nc.dram_tensor("Wp", [128, NT * CO], BF16, kind="ExternalInput")
    uT_d = nc.dram_tensor("uT", [128, NT * B_LOC], BF16, kind="ExternalInput")
    ub_d = nc.dram_tensor("ub", [B_LOC, RI], BF16, kind="ExternalInput")
    E_d = nc.dram_tensor("E", [128, 16], BF16, kind="ExternalInput")
    R8_d = nc.dram_tensor("R8", [16, 128], BF16, kind="ExternalInput")
    o16_d = nc.dram_tensor("o16", [16, 1], BF16, kind="ExternalInput")
    o1x32_d = nc.dram_tensor("o1x32", [1, 32], F32, kind="ExternalInput")
    y_d = nc.dram_tensor("y", [B_LOC, CO], F32, kind="ExternalOutput")

    rg = [list(range(N_CORES))]

    with tile.TileContext(nc) as tc:
        with (
            tc.tile_pool(name="big", bufs=1) as big,
            tc.tile_pool(name="small", bufs=1) as small,
            tc.tile_pool(name="work", bufs=2) as work,
            tc.tile_pool(name="psum", bufs=1, space="PSUM") as psum,
            tc.tile_pool(name="tpsum", bufs=2, space="PSUM") as tpsum,
            tc.tile_pool(name="dram", bufs=8, space="DRAM") as dram,
        ):
            # ---------------- persistent SBUF ----------------
            W_sb = big.tile([128, NT, O, C], BF16, tag="W")
            Wc_sb = big.tile([128, NT, O, C], BF16, tag="Wc")
            P_sb = big.tile([128, NT, O, C], BF16, tag="P")
            uT_sb = big.tile([128, NT, B_LOC], BF16, tag="uT")
            ub_sb = big.tile([B_LOC, RI], BF16, tag="ub")
            Q_sb = big.tile([128, NT * C], BF16, tag="Q")
            H8_sb = big.tile([128, NT, 8, C], BF16, tag="H8")
            H4_sb = big.tile([128, NT, 4, C], BF16, tag="H4")
            H2_sb = big.tile([128, NT, 2, C], BF16, tag="H2")

            E_sb = small.tile([128, 16], BF16, tag="E")
            R8_sb = small.tile([16, 128], BF16, tag="R8")
            o16_sb = small.tile([16, 1], BF16, tag="o16")
            o1x32_sb = small.tile([1, 32], F32, tag="o1x32")
            b_sb = small.tile([16, NT * C], F32, tag="b")    # routing logits
            s_sb = small.tile([B_LOC, CO], F32, tag="s")     # squash scratch
            xn_sb = small.tile([B_LOC, CO], F32, tag="xn")
            ab_sb = small.tile([B_LOC, CO], F32, tag="ab")
            den_sb = small.tile([B_LOC, CO], F32, tag="den")
            num_sb = small.tile([B_LOC, CO], F32, tag="num")
            v_sb = small.tile([B_LOC, CO], F32, tag="v")

            # constants
            nc.sync.dma_start(out=E_sb[:], in_=E_d[:])
            nc.sync.dma_start(out=R8_sb[:], in_=R8_d[:])
            nc.sync.dma_start(out=o16_sb[:], in_=o16_d[:])
            nc.sync.dma_start(out=o1x32_sb[:], in_=o1x32_d[:])
            nc.vector.memset(b_sb[:], 0.0)

            # bulk loads, chunked so iter-1 matmuls can start early
            Wp_v = Wp_d[:].rearrange("p (t f) -> p t f", t=NT)
            uT_v = uT_d[:].rearrange("p (t f) -> p t f", t=NT)
            for ch in range(NT // NCHUNK):
                sl = slice(ch * NCHUNK, (ch + 1) * NCHUNK)
                nc.sync.dma_start(
                    out=W_sb[:, sl, :, :],
                    in_=Wp_v[:, sl, :].rearrange("p t (o c) -> p t o c", o=O))
                nc.sync.dma_start(out=uT_sb[:, sl, :], in_=uT_v[:, sl, :])
            nc.sync.dma_start(out=ub_sb[:], in_=ub_d[:])

            def softmax_and_scale(it):
                """b -> exp(b) -> Wexp; 1/sum(exp) is applied to s columns
                later (softmax denominator commutes through the s-matmul),
                keeping the denominator chain off the critical path."""
                exp_bf = work.tile([16, NT * C], BF16, tag="exp")
                nc.scalar.activation(exp_bf[:], b_sb[:],
                                     mybir.ActivationFunctionType.Exp)
                # denominator branch (consumed only at squash time):
                den_ps = psum.tile([1, NT * C], F32, tag="A1")
                nc.tensor.matmul(den_ps[:, 0:512], o16_sb[:], exp_bf[:, 0:512])
                nc.tensor.matmul(den_ps[:, 512:720], o16_sb[:],
                                 exp_bf[:, 512:720])
                den_c = work.tile([1, C], F32, tag="denc")
                nc.vector.reduce_sum(
                    den_c[:],
                    den_ps[:].rearrange("p (t c) -> p c t", t=NT),
                    axis=mybir.AxisListType.X)
                rden = work.tile([1, C], F32, tag="rden")
                nc.vector.reciprocal(rden[:], den_c[:])
                rden32_ps = psum.tile([B_LOC, C], F32, tag="A1")
                nc.tensor.matmul(rden32_ps[:], o1x32_sb[:], rden[:])
                rden32 = work.tile([B_LOC, C], F32, tag="rden32")
                nc.scalar.copy(rden32[:], rden32_ps[:])
                # replicate exp over i: crep[p=(r8+i), (t,c)] = exp[r, (t,c)]
                crep_ps = tpsum.tile([128, NT * C], F32, tag="T")
                nc.tensor.matmul(crep_ps[:, 0:512], R8_sb[:], exp_bf[:, 0:512])
                nc.tensor.matmul(crep_ps[:, 512:720], R8_sb[:],
                                 exp_bf[:, 512:720])
                crep_bf = work.tile([128, NT * C], BF16, tag="crepbf")
                nc.scalar.copy(crep_bf[:, 0:240], crep_ps[:, 0:240])
                nc.scalar.copy(crep_bf[:, 240:480], crep_ps[:, 240:480])
                nc.scalar.copy(crep_bf[:, 480:720], crep_ps[:, 480:720])
                # Wc = W * c (o broadcast in the middle free dim keeps the
                # innermost c unit-stride -> DVE 2x mode). 3 chunks so the
                # s-chain can start after the first and GpSimd helps.
                crep_v = crep_bf[:].rearrange("p (t c) -> p t c", t=NT)
                for ci, (t0, t1, eng) in enumerate(
                        [(0, 24, nc.vector), (24, 48, nc.vector),
                         (48, 72, nc.vector)]):
                    eng.tensor_mul(
                        Wc_sb[:, t0:t1, :, :],
                        W_sb[:, t0:t1, :, :],
                        crep_v[:, t0:t1, :].unsqueeze(2)
                        .broadcast_to([128, t1 - t0, O, C]))
                return rden32

            def s_chain(it):
                """s[b, co] = uT^T @ (Wc or W), K = 9216 chained."""
                rhs = W_sb if it == 0 else Wc_sb
                s_ps = psum.tile([B_LOC, CO], F32, tag="S0")
                for t in range(NT):
                    nc.tensor.matmul(s_ps[:], uT_sb[:, t, :],
                                     rhs[:, t, :, :],
                                     start=(t == 0), stop=(t == NT - 1))
                return s_ps

            def squash(s_ps, it, rden32=None):
                """v = s*|s|/(1+s^2) elementwise on [b, (o,c)] (fp32).
                The softmax denominator (or the uniform 1/R on iter 0) is
                applied to s columns here."""
                if it == 0:
                    nc.scalar.mul(s_sb[:], s_ps[:], 1.0 / R)
                else:
                    nc.vector.tensor_mul(
                        s_sb[:].rearrange("b (o c) -> b o c", o=O),
                        s_ps[:].rearrange("b (o c) -> b o c", o=O),
                        rden32[:].unsqueeze(1).broadcast_to([B_LOC, O, C]))
                nc.vector.tensor_mul(xn_sb[:], s_sb[:], s_sb[:])
                nc.scalar.activation(ab_sb[:], s_sb[:],
                                     mybir.ActivationFunctionType.Abs)
                nc.scalar.add(den_sb[:], xn_sb[:], 1.0)
                nc.vector.reciprocal(den_sb[:], den_sb[:])
                nc.vector.tensor_mul(num_sb[:], s_sb[:], ab_sb[:])
                if it < NUM_ITERS - 1:
                    v_bf = work.tile([B_LOC, CO], BF16, tag="vbf")
                    nc.vector.tensor_mul(v_bf[:], num_sb[:], den_sb[:])
                    return v_bf
                nc.vector.tensor_mul(v_sb[:], num_sb[:], den_sb[:])
                return None

            def a_phase(v_bf):
                """T = u_b^T @ v per ri-tile (3 tiles per PSUM bank);
                P = W*T batched per bank; Q = sum_o P; a = E^T @ Q."""
                NG = NT // 6
                for g in range(NG):
                    # 6 ri-tiles per 2-bank PSUM tensor (3 per bank, banks
                    # at column offsets 0 and 512)
                    T_ps = tpsum.tile([128, 1024], F32, tag="T")
                    for j in range(6):
                        t = g * 6 + j
                        col = (j // 3) * 512 + (j % 3) * CO
                        nc.tensor.matmul(
                            T_ps[:, col:col + CO],
                            ub_sb[:, t * 128:(t + 1) * 128], v_bf[:])
                    if g % 4 > 0:
                        # ACT copies both banks in one op; the multiply then
                        # runs unit-stride bf16 at 2x on DVE
                        T_cp = work.tile([128, 2, TB * CO], BF16, tag="tcp")
                        nc.scalar.copy(
                            T_cp[:],
                            T_ps[:].rearrange("p (s q) -> p s q", s=2)
                            [:, :, 0:TB * CO])
                        nc.vector.tensor_mul(
                            P_sb[:, g * 6:(g + 1) * 6, :, :]
                            .rearrange("p (s j) o c -> p s j o c", s=2),
                            W_sb[:, g * 6:(g + 1) * 6, :, :]
                            .rearrange("p (s j) o c -> p s j o c", s=2),
                            T_cp[:].rearrange("p s (j o c) -> p s j o c",
                                              j=TB, o=O))
                    else:
                        nc.vector.tensor_mul(
                            P_sb[:, g * 6:(g + 1) * 6, :, :]
                            .rearrange("p (s j) o c -> p s j o c", s=2),
                            W_sb[:, g * 6:(g + 1) * 6, :, :]
                            .rearrange("p (s j) o c -> p s j o c", s=2),
                            T_ps[:].rearrange("p (s q) -> p s q", s=2)
                            [:, :, 0:TB * CO]
                            .rearrange("p s (j o c) -> p s j o c",
                                       j=TB, o=O))
                    if g == NG // 2 - 1:
                        # first-half o-sums overlap the second half's T/P
                        nc.vector.tensor_add(
                            H8_sb[:, 0:NT // 2, :, :],
                            P_sb[:, 0:NT // 2, 0:8, :],
                            P_sb[:, 0:NT // 2, 8:16, :])
                        nc.vector.tensor_add(
                            H4_sb[:, 0:NT // 2, :, :],
                            H8_sb[:, 0:NT // 2, 0:4, :],
                            H8_sb[:, 0:NT // 2, 4:8, :])
                # sum over o: pairwise halving keeps unit-stride c-runs (2x)
                nc.vector.tensor_add(H8_sb[:, NT // 2:NT, :, :],
                                     P_sb[:, NT // 2:NT, 0:8, :],
                                     P_sb[:, NT // 2:NT, 8:16, :])
                nc.vector.tensor_add(H4_sb[:, NT // 2:NT, :, :],
                                     H8_sb[:, NT // 2:NT, 0:4, :],
                                     H8_sb[:, NT // 2:NT, 4:8, :])
                nc.vector.tensor_add(H2_sb[:], H4_sb[:, :, 0:2, :],
                                     H4_sb[:, :, 2:4, :])
                nc.vector.tensor_add(
                    Q_sb[:].rearrange("p (t c) -> p t c", t=NT),
                    H2_sb[:, :, 0, :], H2_sb[:, :, 1, :])
                a_ps = psum.tile([16, NT * C], F32, tag="A1")
                nc.tensor.matmul(a_ps[:, 0:512], E_sb[:], Q_sb[:, 0:512])
                nc.tensor.matmul(a_ps[:, 512:720], E_sb[:], Q_sb[:, 512:720])
                return a_ps

            for it in range(NUM_ITERS):
                rden32 = softmax_and_scale(it) if it > 0 else None
                s_ps = s_chain(it)
                v_bf = squash(s_ps, it, rden32)
                if it < NUM_ITERS - 1:
                    a_ps = a_phase(v_bf)
                    a_sb = work.tile([16, NT * C], BF16, tag="asb")
                    nc.scalar.copy(a_sb[:], a_ps[:])
                    ar_in = dram.tile([16, NT * C], BF16, tag="arin")
                    ar_out = dram.tile([16, NT * C], BF16, tag="arout")
                    nc.sync.dma_start(out=ar_in[:], in_=a_sb[:])
                    nc.gpsimd.collective_compute(
                        "AllReduce", mybir.AluOpType.add,
                        replica_groups=rg,
                        ins=[ar_in.opt()], outs=[ar_out.opt()])
                    a_red = work.tile([16, NT * C], BF16, tag="ared")
                    nc.sync.dma_start(out=a_red[:], in_=ar_out[:])
                    nc.vector.tensor_add(b_sb[:], b_sb[:], a_red[:])
                else:
                    nc.sync.dma_start(out=y_d[:], in_=v_sb[:])

    _legalize_sync_waits(nc)
    return nc


def _host_prep(u, W):
    """Build per-core input maps from full inputs."""
    u = np.ascontiguousarray(np.asarray(u, dtype=np.float32))
    W = np.ascontiguousarray(np.asarray(W, dtype=np.float32))

    W_perm = W[0].transpose(0, 3, 2, 1).reshape(RI, CO)          # [ri, (o,c)]
    Wp = np.ascontiguousarray(
        W_perm.reshape(NT, 128, CO).transpose(1, 0, 2).reshape(128, NT * CO)
    ).astype(bfnp)

    E = np.zeros((128, 16), np.float32)
    E[np.arange(128), np.arange(128) // 8] = 1.0 / B
    E = E.astype(bfnp)
    R8 = np.zeros((16, 128), np.float32)
    R8[np.arange(128) // 8, np.arange(128)] = 1.0
    R8 = R8.astype(bfnp)
    o16 = np.ones((16, 1), np.float32).astype(bfnp)
    o1x32 = np.ones((1, 32), np.float32)

    in_maps = []
    for c in range(N_CORES):
        u_loc = u[c * B_LOC:(c + 1) * B_LOC]                     # [32, R, I]
        u_flat = u_loc.reshape(B_LOC, RI)
        uT = np.ascontiguousarray(
            u_flat.T.reshape(NT, 128, B_LOC).transpose(1, 0, 2)
            .reshape(128, NT * B_LOC)).astype(bfnp)
        ub = np.ascontiguousarray(u_flat).astype(bfnp)
        in_maps.append({
            "Wp": Wp, "uT": uT, "ub": ub, "E": E, "R8": R8,
            "o16": o16, "o1x32": o1x32,
        })
    return in_maps


_cached = {}


def _get_nc():
    if "nc" not in _cached:
        _cached["nc"] = _build_bass()
    return _cached["nc"]


def kernel(u, W, _return_timing=False):
    nc = _get_nc()
    in_maps = _host_prep(u, W)
    res = run_bass_kernel_spmd(
        nc, in_maps, list(range(N_CORES)), trace=_return_timing)
    outs = [res.results[i]["y"].reshape(B_LOC, O, C).transpose(0, 2, 1)
            .reshape(B_LOC, C, O, 1) for i in range(N_CORES)]
    full = np.concatenate(outs, axis=0).astype(np.float32)
    if _return_timing:
        return full, res.exec_time_ns
    return full
